# revision 27
# baseline (speedup 1.0000x reference)
"""Trainium2 Bass kernel for BinOverlapPredictionFromMaxProj (segment max + masked mean).

Full computation:
  ptm: (32, 8, 30, 1, 72, 72) f32, mem_mask: (32, 8, 30) bool
  n = 32*8 = 256 rows; per row: max over 5184-feature axis per mem (30), then
  masked mean over mems -> out (256,) f32.

Sharding: data-parallel over the 256 fused rows across 8 cores (32 rows each).
Per core: 960 segments x 5184 features (~19.9 MB) -> memory-bound.

V3 design notes (on top of the V1 pair-aligned layout):

The 19.9 MB stream rides one SWDGE queue that fans descriptors over the
core's 16 DMA engines. Engine 15 (E79 on nc4) also serves the notification /
HW-dynamic queues and runs ~22% slower in bursts; with uniform round-robin it
builds an ~11us backlog that gates every load's completion semaphore (the
last data landed at ~68us instead of ~57us).

SWDGE lane-assignment rule (measured): for an n-descriptor DMA,
descs_per_lane = smallest divisor of n >= ceil(n/16); nlanes = n/d; lanes
are taken contiguously from a per-queue cursor that advances by nlanes mod
16 and persists across instructions (sem packets don't advance it).
Packet-speed rule (measured): n in {128, 126, 111, 15, 16} -> full 27 GB/s
per engine; n = 8 mod 16 (120, 104, 24) -> ~2x slower per packet; n = 112
(7 descs/lane) -> ~1.4x slower. So only "fast" shapes are used.

Exploits:
  - (126+2) skew pairs: the 126-desc load uses 14 lanes x 9 (full speed,
    skipping lanes cursor+14/15 = engines 14-15), the 2-desc remainder lands
    on those two lanes; advance 14+2 = 16 keeps the cursor parked at 0, so
    the starved lanes are always engines 14/15. 3 of 6 pairs are skewed
    (engine 15 gets ~46 instead of 60 packet-units; its ~22% deficit and
    burst interference fit in the slack), the rest stay uniform 128-desc
    so the fast engine 14 is not over-starved.
  - 16-desc 4-byte-descriptor re-reads of `consts` are all-lane "cover"
    loads: per-lane FIFO means a cover's completion implies every earlier
    descriptor on every lane has drained, so each skew pair's reduce can
    carry ONE sem wait (the cover's) instead of one per writer (walrus
    allows a single attached sync wait). A post-pass rewrites the
    tile-assigned multi-waits accordingly; a tiny DVE copy reads each cover
    tile so the cover gets a completion semaphore at all.

Tail: col 13 is loaded as two full-partition half-columns so the final
delivery-gated reduce is ~1.4us instead of ~2.8us, and the
pairmax/mask/rowsum for segments 0-5 runs mid-stream; only segment 6 and the
final matmul+mean remain after the last byte. 1/count is computed on host
and shipped in m1 (drops the DVE reciprocal and its table load).
"""

import sys

import numpy as np

if "/opt/trn_rl_repo" not in sys.path:
    sys.path.insert(0, "/opt/trn_rl_repo")

NCORES = 8
NF, NS, NMEM, FEAT = 32, 8, 30, 5184
N = NF * NS  # 256
ROWS = N // NCORES  # 32 rows per core
SEGS = ROWS * NMEM  # 960 segments per core
PPART = 128  # partitions
HALF = FEAT // 2  # 2592 floats per half-segment
HPP = SEGS * 2 // PPART  # 15 half-segments per partition
NWHOLE = 7  # whole segments per partition (cols 0..13)
NPAIR = 6  # column-pairs loaded with the skew structure (cols 0..11)
NC_ = PPART + ROWS + NWHOLE  # consts free dim: ident | w1 | maskA

_NC_CACHE = {}


def _nlanes(n):
    """Lanes used by an n-descriptor SWDGE DMA (measured ucode rule)."""
    need = -(-n // 16)
    for d in range(need, n + 1):
        if n % d == 0:
            return n // d
    return 1


def _patch_tile_drain():
    """Split the kernel-tail Drain's semaphore waits into standalone wait_ge
    instructions (one wait per instruction), to fit the walrus per-instruction
    sync-wait limit."""
    import concourse.tile as tile
    from concourse.vector_clock import ScopedClock

    if getattr(tile.TileContext._drain_and_barrier, "_single_wait_patch", False):
        return

    def _drain_and_barrier(self, tick_clock, wait_clock):
        drain_inst = self.nc.sync.drain()
        wait_clock.add_sem_waits(
            drain_inst.ins, ScopedClock({None: tick_clock.global_clock})
        )
        si = drain_inst.ins.sync_info
        waits = list(si.on_wait) if si is not None else []
        if len(waits) > 1:
            si.on_wait = [waits[0]]
            by_name = {h.name: h for h in self.sems.allocated().values()}
            for w in waits[1:]:
                self.nc.sync.wait_ge(by_name[w.ant_name], w.wait_value)

        self.nc.all_engine_barrier()
        assert self.sems is not None
        popped = self.nc._tile_sem_poison_stack.pop()
        assert popped is self._sem_poison
        self.nc.clear_and_free_semaphores(list(self.sems.allocated().values()))

    _drain_and_barrier._single_wait_patch = True
    tile.TileContext._drain_and_barrier = _drain_and_barrier


def _rewrite_cover_waits(nc, cover_map, merge_rest=True):
    """Post-pass: point each recorded reduce's sem waits at its cover load.

    cover_map: list of (cover_dma_bass_inst, [reduce_bass_inst, ...]).
    Per-lane FIFO on the SWDGE queue makes `cover delivered` imply `all
    earlier descriptors on every lane delivered`, so a single wait on the
    cover's accumulated sem value is a sound replacement for the
    tile-assigned one-wait-per-writer set (walrus allows only one).
    """
    # Accumulate DMA completion-sem values in program order.
    acc = {}
    cover_val = {}
    cover_ids = {id(c.ins): c for c, _ in cover_map}
    for fn in nc.m.functions:
        for b in fn.blocks:
            for ins in b.instructions:
                if type(ins).__name__ != "InstDMACopy":
                    continue
                si = ins.sync_info
                if si is None or not si.on_update:
                    continue
                for u in si.on_update:
                    if not u.ant_name.startswith("DMASW"):
                        continue
                    acc[u.id] = acc.get(u.id, 0) + u.update_value
                    if id(ins) in cover_ids:
                        cover_val[id(ins)] = (u.ant_name, u.id, acc[u.id])
    for cover, reds in cover_map:
        key = id(cover.ins)
        assert key in cover_val, f"cover {cover.ins.name} got no DMASW sem"
        ant_name, sem_id, val = cover_val[key]
        for r in reds:
            si = r.ins.sync_info
            assert si is not None and si.on_wait, f"{r.ins.name} has no waits"
            w = si.on_wait[0]
            w.ant_name = ant_name
            w.id = sem_id
            w.wait_value = val
            si.on_wait = [w]
    if merge_rest:
        eng_prefix = {"DVE": "DVE_", "Pool": "Pool_", "Activation": "Act_",
                      "PE": "PE_", "SP": "SP_"}
        for fn in nc.m.functions:
            for b in fn.blocks:
                for ins in b.instructions:
                    si = ins.sync_info
                    if si is None or len(si.on_wait) <= 1:
                        continue
                    waits = list(si.on_wait)
                    # An engine's wait on its OWN engine-sem is satisfied by
                    # program order (the incrementer is an earlier
                    # instruction on the same serial engine): drop it when
                    # other waits remain.
                    own = eng_prefix.get(
                        getattr(getattr(ins, "engine", None), "name", ""), "@@"
                    )
                    rest = [w for w in waits if not w.ant_name.startswith(own)]
                    if rest:
                        waits = rest
                    names = {w.ant_name for w in waits}
                    assert len(names) == 1, (
                        f"unresolved multi-sem wait on {ins.name}: "
                        f"{[(w.ant_name, w.wait_value) for w in waits]}"
                    )
                    best = max(waits, key=lambda w: w.wait_value)
                    si.on_wait = [best]


def _build_nc():
    import concourse.bass as bass
    import concourse.tile as tile
    from concourse import mybir
    from concourse.bass import MemorySpace

    _patch_tile_drain()

    f32 = mybir.dt.float32
    X = mybir.AxisListType.X

    nc = bass.Bass("TRN2")
    ptm = nc.dram_tensor("ptm", [PPART, HPP, HALF], f32, kind="ExternalInput")
    consts = nc.dram_tensor("consts", [PPART, NC_], f32, kind="ExternalInput")
    m1 = nc.dram_tensor("m1", [1, 2 * ROWS + 1], f32, kind="ExternalInput")
    out = nc.dram_tensor("out", [1, ROWS], f32, kind="ExternalOutput")

    cover_map = []  # (cover dma inst, [reduces to rewrite])
    cursor = 0  # SWDGE lane cursor (relative; engine 15 = cursor 15)

    def q0(dst, src, ndesc, expect_adv):
        nonlocal cursor
        inst = nc.gpsimd.dma_start(out=dst, in_=src)
        adv = _nlanes(ndesc) % 16
        assert adv == expect_adv % 16, (ndesc, adv, expect_adv)
        cursor = (cursor + adv) % 16
        return inst

    with tile.TileContext(nc) as tc:
        with (
            tc.tile_pool(name="data", bufs=1) as dpool,
            tc.tile_pool(name="small", bufs=1) as spool,
            tc.tile_pool(name="psum", bufs=1, space=MemorySpace.PSUM) as ppool,
        ):
            HH = HALF // 2  # 1296 floats per half-column slice
            H4 = HALF // 4

            # --- early loads (CONST is issued after the stream's first
            # quarter-column so the DVE's first reduce starts sooner) -------
            const_t = spool.tile([PPART, NC_], f32)
            ident_v = const_t[:, 0:PPART]
            w1_v = const_t[:, PPART : PPART + ROWS]
            maskA_v = const_t[:, PPART + ROWS : NC_]

            m1_t = spool.tile([1, 2 * ROWS + 1], f32)
            nc.scalar.dma_start(out=m1_t[:], in_=m1[:])
            maskS2_v = m1_t[0:1, 0 : 2 * ROWS]  # pre-scaled by 1/count
            one_v = m1_t[0:1, 2 * ROWS : 2 * ROWS + 1]

            # Mixed-granularity stream, all 128-descriptor loads (the only
            # shape that sustains full per-engine DMA speed): a quarter-col
            # first so the DVE starts reducing at ~9.5us, full columns
            # mid-stream (fewest packets), halves/quarters again for the
            # last columns so the end-of-stream reduce backlog stays tiny.
            # Stream order: stray col 0 (q,q,h), cols 1..10 full,
            # cols 11..13 halves, col 14 half + two quarters.
            dS = dpool.tile([PPART, 1, HALF], f32, name="dataS", tag="dataS")
            q0(dS[:, :, 0:H4], ptm[:, 0:1, 0:H4], 128, 0)
            q0(const_t[:], consts[:], 128, 0)
            # PE warmups: touch const and m1 tiles (after their loads in
            # program order) so later PE ops carry a single data wait each
            # (walrus one-wait limit).
            warm = ppool.tile([1, ROWS], f32)
            nc.tensor.matmul(warm[:], const_t[:, 0:1], const_t[:, 0:ROWS],
                             start=True, stop=True)
            warm2 = ppool.tile([1, ROWS], f32)
            nc.tensor.matmul(warm2[:], m1_t[0:1, 0:1], m1_t[0:1, 0:ROWS],
                             start=True, stop=True)
            q0(dS[:, :, H4:HH], ptm[:, 0:1, H4:HH], 128, 0)
            q0(dS[:, :, HH:HALF], ptm[:, 0:1, HH:HALF], 128, 0)
            sq1 = spool.tile([PPART, 1], f32)
            nc.vector.reduce_max(out=sq1[:], in_=dS[:, :, 0:H4], axis=X)
            sq2 = spool.tile([PPART, 1], f32)
            nc.vector.reduce_max(out=sq2[:], in_=dS[:, :, H4:HH], axis=X)

            # DVE warm-touch + power-ramp burn in the pre-stream window.
            touch = spool.tile([1, 1], f32)
            nc.vector.tensor_copy(out=touch[:], in_=const_t[0:1, 0:1])
            touch2 = spool.tile([1, 1], f32)
            nc.vector.tensor_copy(out=touch2[:], in_=m1_t[0:1, 0:1])
            burn = spool.tile([PPART, 1], f32)
            nc.vector.reduce_max(out=burn[:], in_=const_t[:], axis=X)
            nc.vector.reduce_max(out=burn[:], in_=const_t[:], axis=X)

            sq = spool.tile([PPART, 1], f32)
            nc.vector.tensor_max(out=sq[:], in0=sq1[:], in1=sq2[:])
            sh = spool.tile([PPART, 1], f32)
            nc.vector.reduce_max(out=sh[:], in_=dS[:, :, HH:HALF], axis=X)
            statS = spool.tile([PPART, 1], f32)
            nc.vector.tensor_max(out=statS[:], in0=sq[:], in1=sh[:])

            # PE transpose of the stray half-maxes to one partition.
            strayP = ppool.tile([1, PPART], f32)
            nc.tensor.transpose(strayP[:], statS[:], ident_v)

            # stats: 2 half-stats per column for ptm cols 1..14.
            m14 = spool.tile([PPART, 2 * NWHOLE], f32)
            stats = spool.tile([PPART, 4 * NWHOLE], f32)
            straysum = spool.tile([1, ROWS], f32)
            qa = spool.tile([PPART, 1], f32)
            qb = spool.tile([PPART, 1], f32)

            for c in range(1, 11):
                d = dpool.tile([PPART, 1, HALF], f32, name=f"dC{c}", tag=f"dC{c}")
                assert cursor == 0, cursor
                q0(d[:, :, 0:HH], ptm[:, c : c + 1, 0:HH], 128, 0)
                q0(d[:, :, HH:HALF], ptm[:, c : c + 1, HH:HALF], 128, 0)
                sc = 2 * (c - 1)
                nc.vector.reduce_max(
                    out=stats[:, sc : sc + 1], in_=d[:, :, 0:HH], axis=X
                )
                nc.vector.reduce_max(
                    out=stats[:, sc + 1 : sc + 2], in_=d[:, :, HH:HALF], axis=X
                )

                if c == 2:
                    # Stray path on DVE, mid-stream (off critical path).
                    strayC = spool.tile([1, PPART], f32)
                    nc.vector.tensor_copy(out=strayC[:], in_=strayP[:])
                    strayM = spool.tile([1, 2 * ROWS], f32)
                    nc.vector.tensor_max(
                        out=strayM[:],
                        in0=strayC[0:1, 0:PPART:2],
                        in1=strayC[0:1, 1:PPART:2],
                    )
                    strayMM = spool.tile([1, 2 * ROWS], f32)
                    nc.vector.tensor_mul(
                        out=strayMM[:], in0=strayM[:], in1=maskS2_v
                    )
                    sv = strayMM[:].rearrange("one (r two) -> one r two", two=2)
                    nc.vector.reduce_sum(out=straysum[:], in_=sv, axis=X)

            for c in range(11, 14):
                d = dpool.tile([PPART, 1, HALF], f32, name=f"dC{c}", tag=f"dC{c}")
                assert cursor == 0, cursor
                q0(d[:, :, 0:HH], ptm[:, c : c + 1, 0:HH], 128, 0)
                q0(d[:, :, HH:HALF], ptm[:, c : c + 1, HH:HALF], 128, 0)
                sc = 2 * (c - 1)
                nc.vector.reduce_max(
                    out=stats[:, sc : sc + 1], in_=d[:, :, 0:HH], axis=X
                )
                nc.vector.reduce_max(
                    out=stats[:, sc + 1 : sc + 2], in_=d[:, :, HH:HALF], axis=X
                )

            # Col 14: first half, then two quarters (smallest pieces last).
            d14 = dpool.tile([PPART, 1, HALF], f32, name="dC14", tag="dC14")
            assert cursor == 0, cursor
            q0(d14[:, :, 0:HH], ptm[:, 14:15, 0:HH], 128, 0)
            q0(d14[:, :, HH : HH + H4], ptm[:, 14:15, HH : HH + H4], 128, 0)
            q0(d14[:, :, HH + H4 : HALF], ptm[:, 14:15, HH + H4 : HALF], 128, 0)
            nc.vector.reduce_max(out=stats[:, 26:27], in_=d14[:, :, 0:HH], axis=X)
            nc.vector.reduce_max(out=qa[:], in_=d14[:, :, HH : HH + H4], axis=X)
            nc.vector.reduce_max(out=qb[:], in_=d14[:, :, HH + H4 : HALF], axis=X)
            nc.vector.tensor_max(out=stats[:, 27:28], in0=qa[:], in1=qb[:])
            nc.vector.tensor_max(
                out=m14[:],
                in0=stats[:, 0 : 4 * NWHOLE : 2],
                in1=stats[:, 1 : 4 * NWHOLE : 2],
            )

            # Segment maxes: stride-2 max over per-column maxes, then mask
            # (rcnt folded on host), row partial, matmul + stray matmul
            # accumulated in PSUM, copy out.
            seg = spool.tile([PPART, NWHOLE], f32)
            nc.vector.tensor_max(
                out=seg[:],
                in0=m14[:, 0 : 2 * NWHOLE : 2],
                in1=m14[:, 1 : 2 * NWHOLE : 2],
            )
            masked = spool.tile([PPART, NWHOLE], f32)
            nc.vector.tensor_mul(out=masked[:], in0=seg[:], in1=maskA_v)
            partial = spool.tile([PPART, 1], f32)
            nc.vector.reduce_sum(out=partial[:], in_=masked[:], axis=X)

            acc = ppool.tile([1, ROWS], f32)
            nc.tensor.matmul(acc[:], partial[:], w1_v, start=True, stop=False)
            nc.tensor.matmul(acc[:], one_v, straysum[:], start=False, stop=True)

            res = spool.tile([1, ROWS], f32)
            nc.vector.tensor_copy(out=res[:], in_=acc[:])
            nc.scalar.dma_start(out=out[:], in_=res[:])

    _rewrite_cover_waits(nc, cover_map, merge_rest=True)
    return nc


def _get_nc():
    if "nc" not in _NC_CACHE:
        _NC_CACHE["nc"] = _build_nc()
    return _NC_CACHE["nc"]


def _host_layout():
    """Pair-aligned half-segment permutation and mask/weight constants.

    idx[p, j] = half-segment index (seg*2 + half, within one core's 1920)
    placed at (partition p, col j). Row r owns partitions 4r..4r+3; each
    holds 7 whole segments (cols 0..13, halves adjacent) plus one stray
    half at col 14 (segs 28/29 of the row, halves on partition pairs).
    """
    idx = np.empty((PPART, HPP), dtype=np.int64)
    w1row = np.zeros((PPART, ROWS), dtype=np.float32)
    for r in range(ROWS):
        for j in range(4):
            p = 4 * r + j
            w1row[p, r] = 1.0
            for k in range(NWHOLE):
                seg = r * NMEM + 7 * j + k
                idx[p, 1 + 2 * k] = 2 * seg
                idx[p, 2 + 2 * k] = 2 * seg + 1
        idx[4 * r + 0, 0] = 2 * (r * NMEM + 28)
        idx[4 * r + 1, 0] = 2 * (r * NMEM + 28) + 1
        idx[4 * r + 2, 0] = 2 * (r * NMEM + 29)
        idx[4 * r + 3, 0] = 2 * (r * NMEM + 29) + 1
    ident = np.eye(PPART, dtype=np.float32)
    return idx.reshape(-1), w1row, ident


_IDX, _W1ROW, _IDENT = _host_layout()


def make_in_maps(ptm, mem_mask):
    ptm = np.ascontiguousarray(np.asarray(ptm, dtype=np.float32))
    mask = np.asarray(mem_mask).reshape(N, NMEM).astype(np.float32)
    halves = ptm.reshape(N * NMEM * 2, HALF)

    in_maps = []
    for i in range(NCORES):
        core_halves = halves[i * SEGS * 2 : (i + 1) * SEGS * 2]
        shard = core_halves[_IDX].reshape(PPART, HPP, HALF)
        m = mask[i * ROWS : (i + 1) * ROWS]  # (32, 30)
        rcnt = (1.0 / m.sum(axis=1)).astype(np.float32)
        maskA = np.empty((PPART, NWHOLE), dtype=np.float32)
        for j in range(4):
            maskA[j::4] = m[:, 7 * j : 7 * j + 7]
        consts = np.concatenate([_IDENT, _W1ROW * rcnt[None, :], maskA], axis=1)
        consts = np.ascontiguousarray(consts, dtype=np.float32)
        # m1 = maskS2 * (1/count) | 1.0, on one partition; 1/count is also
        # folded into the w1 matmul weights so the masked mean needs no
        # separate multiply on device.
        m1 = np.concatenate(
            [(m[:, 28:30] * rcnt[:, None]).reshape(-1), np.ones(1, np.float32)]
        ).reshape(1, -1)
        in_maps.append(
            {
                "ptm": shard,
                "consts": consts,
                "m1": np.ascontiguousarray(m1.astype(np.float32)),
            }
        )
    return in_maps


def _ensure_ntff_hook():
    """Register the axon NTFF profiling hook (the container's antenv lacks
    axon_hooks; synthesize it from trn_agent_boot), and stub the artifact
    upload which has no bucket access here."""
    import types

    try:
        from antenv.axon_hooks import get_axon_ntff_profile_hook  # noqa: F401
    except ImportError:
        import antenv
        from trn_agent_boot.trn_boot import _ntff_profile_via_ctypes

        mod = types.ModuleType("antenv.axon_hooks")
        mod._hook = _ntff_profile_via_ctypes("/opt/axon/libaxon_pjrt.so")
        mod.set_axon_ntff_profile_hook = lambda h: setattr(mod, "_hook", h)
        mod.get_axon_ntff_profile_hook = lambda: mod._hook
        sys.modules["antenv.axon_hooks"] = mod
        antenv.axon_hooks = mod

    from concourse import bass_utils

    if not getattr(bass_utils.upload_artifacts, "_stubbed", False):
        def _no_upload(tmpdir):
            return str(tmpdir)

        _no_upload._stubbed = True
        bass_utils.upload_artifacts = _no_upload


def run(ptm, mem_mask, trace=False):
    from concourse.bass_utils import run_bass_kernel_spmd

    if trace:
        _ensure_ntff_hook()

    in_maps = make_in_maps(ptm, mem_mask)

    nc = _get_nc()
    kr = run_bass_kernel_spmd(nc, in_maps, list(range(NCORES)), trace=trace)
    out = np.concatenate(
        [np.asarray(kr.results[i]["out"]).reshape(ROWS) for i in range(NCORES)]
    )
    return out.astype(np.float32), kr


def kernel(ptm, mem_mask):
    out, _ = run(ptm, mem_mask, trace=False)
    return out
